# revision 72
# baseline (speedup 1.0000x reference)
"""KAN layer kernel for trn2 (8 NeuronCores, SPMD data-parallel over tokens).

Math: reference computes, per element x with t = tanh(x):
  out[n,o] = sum_i W[o,i] * (c0_i*B0(t_ni) + c1_i*B1(t_ni))
with B0/B1 cubic B-splines on knots linspace(-1,1,8); only the first 2 of 5
coeff columns are active.  6*B_k(t) = rho^3 - 4 sigma^3 with
  q = |3.5t + b_k| (b_0=1.5, b_1=0.5), rho = relu(2-q), sigma = relu(1-q).

Identity: with n = min(q-2, 0) (= -rho) and D = 4^(1/3):
  E_k = rho^3 - 4 sigma^3 = min(D*(n+1), 0)^3 - n^3        (KAN_EHATN, 8-stage)
so the ACT-route per basis is: q = Abs(3.5t+b) on Act (1 LUT pass), then
n = (q-2) min 0 as a 4x-mode tensor_scalar on DVE (0.26 ns/col), then one
8-stage custom DVE op.  The DVE-route (for balance) is the classic
KAN_HAT (rho from t, 4-stage) -> KAN_EHAT (8-stage) pair, all on DVE.

Hardware legality notes (found the hard way; TimelineSim does NOT check):
  - GPSIMD/Pool cannot access PSUM, and in fact cannot run TensorScalar at
    all on real HW (neuron_isa_check_opcode_on_engine) -> Pool is unusable
    for elementwise work; only Act+DVE carry the kernel.
  - abs_max is not a valid tensor_scalar ALU op on HW (any position).
  - bass dma_start cannot read PSUM -> drains must be Act/DVE copies.
  - Splitting a PSUM accumulation group (start..stop) into two passes over
    the same region (k-split) silently corrupts results on HW.

Engine balance (per-core busy, ~52-54us each):
  Act:  tanh + Abs-q for ACT-route bases + most PSUM drains
  DVE:  n-passes + KAN_EHATN + HAT/EHAT for DVE_Q bases + tail drains
  PE:   out[tok,o] = sum_kh E_k[h]^T @ M_kT[h]  (bf16, K=512, batched bursts)
  DMA:  bf16 in [ch,tok]; bf16 out
host:  x -> bf16, transpose to [ch, tok]; M_kT[i,o] = W[o,i]*c_k[i]/6 (bf16)

Slices sized [512,1024*6,512,512,256,256] (small ends shorten the
startup/tail serial chains); slice 0 is fully DVE-routed so DVE starts
right after the first tanh (~4.9us) instead of waiting for Act's q passes;
MIXED slices route basis 0 via DVE and basis 1 via Act for balance.  TimelineSim: 63147 ns (baseline 73260).
"""

import sys

sys.path.insert(0, "/opt/trn_rl_repo")

import numpy as np
import ml_dtypes

D_CBRT4 = 4.0 ** (1.0 / 3.0)

N_CORES = 8
TOK_TOTAL = 16 * 4096
TOK = TOK_TOTAL // N_CORES  # 8192 tokens per core
IN_DIM = 256
OUT_DIM = 256
# variable slice sizes: small first slice (DVE starts sooner) and small last
# slice (short matmul/drain/DMA tail)
CTS = [512, 1024, 1024, 1024, 1024, 1024, 1024, 512, 512, 256, 256]
assert sum(CTS) == TOK
# slices whose n1 pass runs on Pool (early-middle: Pool latency absorbed)
POOL_N1 = set()
# slices whose q-pair is produced on DVE via u-chain (Act is loaded with
# drains; ~40% of q work moves to DVE for balance)
DVE_Q = {0, 2, 4, 6}
DVE_DRAIN = {9, 10}
MIXED = {4, 6, 8}  # basis 1 Act-routed even in DVE_Q slices
# flush batches (PE p-state: bursts of ~2 slices); last gets the k-split
BATCHES = [(0, 1), (2, 3, 4), (5, 6, 7), (8, 9, 10)]

_CACHE = {}


def _register_ops():
    from concourse import dve_ops
    from concourse.dve_ops import DveOp, OPS, CUSTOM_DVE_SPECS
    from concourse.dve_spec import (
        Spec, Src0, C0, C1, C2, Zero, One, sq, minn, lower, Bin, _has_src1,
    )
    from concourse.dve_uop import DveOpSpec, AluOp

    def make(name, spec):
        if name in dve_ops._SUB_OPCODE_FOR_NAME:
            return next(op for op in OPS if op.name == name)
        row = dve_ops._CUSTOM_DVE_ROW_BASE + len(OPS)
        assert row < 0x20
        dve_ops._SUB_OPCODE_FOR_NAME[name] = row
        shas = {}
        for ver in ("v3", "v4"):
            tmp = DveOpSpec(
                name=name, opcode=row, uops=lower(spec, ver=ver),
                rd1_en=_has_src1(spec),
            )
            shas[ver] = tmp.sha(ver)
        op = DveOp(name, spec, subdim=False, uops_sha=shas)
        OPS.append(op)
        CUSTOM_DVE_SPECS[name] = spec
        return op

    # KAN_EHATN: from n = min(q-2, 0) (= -rho):
    #   E = min(C0*(n+1), 0)^3 - n^3   with C0 = 4^(1/3)
    n = Src0
    a = n + One
    b = a * C0
    z = minn(b, Zero)
    z3 = z * sq(z)
    n3 = n * sq(n)
    body = z3 - n3

    def ehatn_ref(in0, in1, s0, s1, imm2):
        z = np.minimum(s0 * (in0 + 1.0), 0.0)
        return (z * np.square(z) - in0 * np.square(in0)).astype(np.float32)

    KAN_EHATN = make("KAN_EHATN", Spec(body=body, reference=ehatn_ref))

    # KAN_HAT: rho = C2 - min(|Src0*C0 - C1|, C2) = relu(2-q) for C2=2
    m = Src0 * C0
    qq = Bin(AluOp.ABSOLUTE_DIFF, m, C1)
    c = minn(qq, C2)
    hat_body = C2 - c

    def hat_ref(in0, in1, s0, s1, imm2):
        qv = np.abs(in0 * s0 - s1)
        return (imm2 - np.minimum(qv, imm2)).astype(np.float32)

    KAN_HAT = make("KAN_HAT", Spec(body=hat_body, reference=hat_ref))

    # KAN_EHAT: from rho: E = rho^3 + min(CP*(rho-C1), 0)^3, CP=-cbrt(4), C1=1
    A = Src0
    t1 = A * sq(A)
    B = (A - C1) * C0
    zz2 = minn(B, Zero)
    t2 = zz2 * sq(zz2)
    ehat_body = t1 + t2

    def ehat_ref(in0, in1, s0, s1, imm2):
        t1 = in0 * np.square(in0)
        zv = np.minimum((in0 - s1) * s0, 0.0)
        return (t1 + zv * np.square(zv)).astype(np.float32)

    KAN_EHAT = make("KAN_EHAT", Spec(body=ehat_body, reference=ehat_ref))
    return KAN_EHATN, KAN_HAT, KAN_EHAT


def _build_bass():
    import concourse.bass as bass
    import concourse.bacc as bacc
    import concourse.mybir as mybir
    from concourse import tile

    KAN_EHATN, KAN_HAT, KAN_EHAT = _register_ops()

    f32 = mybir.dt.float32
    bf16 = mybir.dt.bfloat16
    Alu = mybir.AluOpType
    Tanh = mybir.ActivationFunctionType.Tanh
    Abs = mybir.ActivationFunctionType.Abs
    Copy = mybir.ActivationFunctionType.Copy

    nc = bacc.Bacc(None, target_bir_lowering=False)

    xt = nc.dram_tensor("xt", [IN_DIM, TOK], bf16, kind="ExternalInput")
    w0 = nc.dram_tensor("w0", [IN_DIM, OUT_DIM], bf16, kind="ExternalInput")
    w1 = nc.dram_tensor("w1", [IN_DIM, OUT_DIM], bf16, kind="ExternalInput")
    out = nc.dram_tensor("out", [TOK, OUT_DIM], bf16, kind="ExternalOutput")

    BS = (1.5, 0.5)  # b_k per basis

    with tile.TileContext(nc) as tc:
        with (
            tc.tile_pool(name="const", bufs=1) as cpool,
            tc.tile_pool(name="sbuf", bufs=2) as pool,
            tc.tile_pool(name="psum", bufs=4, space="PSUM") as ppool,
        ):
            wt = []
            for k, wk in enumerate((w0, w1)):
                row = []
                for h in range(2):
                    w = cpool.tile([128, OUT_DIM], bf16, tag=f"w{k}{h}")
                    nc.gpsimd.dma_start(w[:], wk[h * 128:(h + 1) * 128, :])
                    row.append(w)
                wt.append(row)
            bias_t = {}
            for bv in BS:
                bt = cpool.tile([128, 1], f32, tag=f"bias{bv}")
                nc.vector.memset(bt[:], bv)
                bias_t[bv] = bt

            offs = [sum(CTS[:i]) for i in range(len(CTS))]
            nslice = len(CTS)

            def flush(pend, ksplit):
                # matmul/drain/DMA for pending slices back-to-back so PE runs
                # one long burst (p-state ramps once, not per slice).  With
                # ksplit, the last slice's k=0 matmuls are emitted before its
                # k=1 matmuls across all groups, so PE overlaps the final
                # KAN_EHATN (E1) on DVE.
                for i, (ps_, pes) in enumerate(pend):
                    ct = CTS[ps_]
                    gsz = min(ct, 512)
                    for g in range(ct // gsz):
                        acc = ppool.tile([128, 1024], f32, tag="acc")
                        for j in range(gsz // 128):
                            col = (g * 4 + j) * 128
                            mm = 0
                            for k in range(2):
                                for h in range(2):
                                    nc.tensor.matmul(
                                        acc[:, j * 256:(j + 1) * 256],
                                        pes[k][:, h * ct + col:h * ct + col + 128],
                                        wt[k][h][:],
                                        start=(mm == 0),
                                        stop=(mm == 3),
                                    )
                                    mm += 1
                        wos = gsz * 2
                        os_t = pool.tile([128, 1024], bf16, tag="os", bufs=4)
                        if ps_ in DVE_DRAIN:
                            nc.vector.tensor_scalar(
                                os_t[:, :wos], acc[:, :wos], 0.0, None, Alu.add)
                        else:
                            nc.scalar.activation(os_t[:, :wos], acc[:, :wos], Copy)
                        o0 = offs[ps_] + g * gsz
                        ovg = out[o0:o0 + gsz, :]
                        nc.sync.dma_start(
                            ovg.rearrange("(a p) o -> p a o", p=128),
                            os_t[:, :wos].rearrange("p (a o) -> p a o", o=OUT_DIM),
                        )

            pend = []
            for s in range(nslice):
                ct = CTS[s]
                w2 = 2 * ct
                xs = pool.tile([128, w2], bf16, tag=f"xs{ct}", bufs=2)
                nc.sync.dma_start(
                    xs[:].rearrange("p (h t) -> p h t", h=2),
                    xt.rearrange("(h p) t -> p h t", h=2)[:, :, offs[s]:offs[s] + ct],
                )
                t = pool.tile([128, w2], bf16, tag=f"t{ct}", bufs=3)
                nc.scalar.activation(t[:], xs[:], Tanh)

                last = s == nslice - 1
                qs = [None, None]
                es = []
                for k in range(2):
                    if s in DVE_Q and not (s in MIXED and k == 1):
                        # all-DVE route: rho via KAN_HAT, E via KAN_EHAT
                        rk = pool.tile([128, w2], bf16, tag=f"r{k}{ct}", bufs=3)
                        nc.vector._custom_dve(
                            KAN_HAT, out=rk[:], in0=t[:],
                            s0=3.5, s1=-BS[k], imm2=2.0)
                        Ek = pool.tile([128, w2], bf16, tag=f"E{k}{ct}", bufs=3)
                        nc.vector._custom_dve(
                            KAN_EHAT, out=Ek[:], in0=rk[:],
                            s0=-D_CBRT4, s1=1.0, imm2=0.0)
                        es.append(Ek)
                        continue
                    if True:
                        qk = pool.tile([128, w2], bf16, tag=f"q{k}{ct}", bufs=3)
                        nc.scalar.activation(
                            qk[:], t[:], Abs, bias=bias_t[BS[k]][:], scale=3.5)
                        nk = pool.tile([128, w2], bf16, tag=f"n{k}{ct}", bufs=3)
                        nc.vector.tensor_scalar(
                            nk[:], qk[:], 2.0, 0.0, Alu.subtract, Alu.min)
                        Ek = pool.tile([128, w2], bf16, tag=f"E{k}{ct}", bufs=3)
                        nc.vector._custom_dve(
                            KAN_EHATN, out=Ek[:], in0=nk[:], s0=D_CBRT4)
                        es.append(Ek)

                pend.append((s, es))
                if s in [b[-1] for b in BATCHES]:
                    flush(pend, ksplit=last)
                    pend = []

    nc.compile()
    return nc


def _get_nc():
    if "nc" not in _CACHE:
        _CACHE["nc"] = _build_bass()
    return _CACHE["nc"]


def kernel(x, inner_coeffs, outer_coeffs):
    from concourse import bass_utils

    bf16 = ml_dtypes.bfloat16
    x = np.asarray(x, dtype=np.float32)
    inner = np.asarray(inner_coeffs, dtype=np.float32)
    outer = np.asarray(outer_coeffs, dtype=np.float32)

    B, S, I = x.shape
    # [tok, ch] -> bf16 -> transpose to [ch, tok] (channel-major on device)
    xbf = x.reshape(B * S, I).astype(bf16)
    xT = np.ascontiguousarray(xbf.T)  # [256, TOK_TOTAL]

    # M_kT[i,o] = W[o,i] * c_k[i] / 6
    m0 = ((outer.T * inner[:, 0:1]) / 6.0).astype(bf16)
    m1 = ((outer.T * inner[:, 1:2]) / 6.0).astype(bf16)

    nc = _get_nc()
    in_maps = []
    for i in range(N_CORES):
        in_maps.append({
            "xt": np.ascontiguousarray(xT[:, i * TOK:(i + 1) * TOK]),
            "w0": m0, "w1": m1,
        })
    res = bass_utils.run_bass_kernel_spmd(nc, in_maps, list(range(N_CORES)))
    outs = [np.asarray(res.results[i]["out"], dtype=np.float32)
            for i in range(N_CORES)]
    full = np.concatenate(outs, axis=0).reshape(B, S, OUT_DIM)
    return full
